# revision 29
# baseline (speedup 1.0000x reference)
"""Sparse attention (template/search) Trainium2 Bass kernel.

Reference computation (B=64, N=320, C=768, H=12, D=64, num_t=64, num_s=256):
    qkv = x @ w_qkv.T + b_qkv           -> split to q, k, v per head
    template tokens 0:64   attend to tokens 0:64
    search   tokens 64:320 attend to all 320 tokens
    out = attn_out @ w_proj.T + b_proj

Strategy: data-parallel over batch across 8 NeuronCores (8 batches each).
All layout transposes happen on the host (numpy):
  - x is fed transposed (xT [768, 320] per batch) so the contraction dim is on
    SBUF partitions for the qkv projection.
  - w_qkv/w_proj are fed transposed; the v-weights are interleaved per head
    with an extra "ones" column (stride 65) so the PV matmul produces the
    softmax denominators in the same PSUM tile as the attention output.
On-device dataflow per (batch, head):
  STk   = kT[d, kchunk].T @ qT[d, :]        (scores transposed, k on partitions)
  PT    = exp(STk * 0.125)                  (ScalarE, PSUM -> SBUF, fp32r)
  PV    = vaug[k, 65].T @ PT[k, q]          -> [65, 320]: rows 0:64 = attn outT,
                                               row 64 = colsums (ones column)
Normalization is deferred per batch: the 12 heads' colsums are gathered into
one [12, 320] tile, one batched reciprocal, then 12 consecutive PE rank-1
broadcasts (shared stationary operand) + in-place multiplies.
Projection: out[t, co] = aT[c, t].T @ w_projT[c, co] (+bias via rank-1 matmul).
Matmuls use float32r (full PE rate at N>=256, ~1e-4 rel err vs fp32); the
q/k score operands are bf16 (~1e-3 overall rel err) which halves their weight
loads and SBUF so the qkT tiles can double-buffer across batch pairs.
Batches are processed in pairs (qkv weights stationary across 2 matmuls), and
the whole schedule is software-pipelined: pair p+1's dense qkv matmuls are
interleaved into pair p's attention phase to keep the PE array duty above the
HAM clock-gate threshold (the PE runs at 1.2 GHz instead of 2.4 when its duty
drops for a ~3.4us window).
"""

import sys

sys.path.insert(0, "/opt/trn_rl_repo")

import numpy as np

B, N, C = 64, 320, 768
H, D = 12, 64
NT, NS = 64, 256
NCORES = 8
BC = B // NCORES  # batches per core
CCH = C // 128  # 6 contraction chunks
QK_TILES = (2 * C) // 128  # 12 co-tiles covering q and k sections
TCH = [(0, 128), (128, 128), (256, 64)]  # token chunks (t or k)
VW = H * 65  # 780: v width incl. ones columns
NPH = VW // 2  # 390: vnat free-dim half
PH = C // 2  # 384: proj free-dim half

_CACHE = {}


def _build():
    import concourse.bacc as bacc
    import concourse.mybir as mybir
    import concourse.tile as tile

    F32 = mybir.dt.float32
    F32R = mybir.dt.float32r
    BF16 = mybir.dt.bfloat16
    EXP = mybir.ActivationFunctionType.Exp

    nc = bacc.Bacc("TRN2")

    d_xt = nc.dram_tensor("xt", [BC, C, N], F32R, kind="ExternalInput")
    d_wqk = nc.dram_tensor("wqk", [C, 2 * C], F32R, kind="ExternalInput")
    d_wv = nc.dram_tensor("wv", [C, VW], F32R, kind="ExternalInput")
    d_wp = nc.dram_tensor("wp", [C, C], F32R, kind="ExternalInput")
    d_bqk = nc.dram_tensor("bqk", [128, QK_TILES], F32, kind="ExternalInput")
    d_bv = nc.dram_tensor("bv", [1, VW], F32R, kind="ExternalInput")
    d_bp = nc.dram_tensor("bp", [1, C], F32R, kind="ExternalInput")
    d_ones = nc.dram_tensor("ones", [1, 128], F32R, kind="ExternalInput")
    d_out = nc.dram_tensor("out", [BC, N, C], F32, kind="ExternalOutput")

    with tile.TileContext(nc) as tc:
        with (
            tc.tile_pool(name="const", bufs=1) as cp,
            tc.tile_pool(name="work", bufs=2) as wp,
            tc.tile_pool(name="psum", bufs=2, space="PSUM") as pp,
        ):
            # ---- resident weights ----
            wqk_sb = []
            wv_sb = []
            wp_sb = []
            for c in range(CCH):
                t_wqk = cp.tile([128, 2 * C], F32R, name=f"wqk{c}", tag=f"wqk{c}")
                nc.sync.dma_start(t_wqk[:], d_wqk[c * 128 : (c + 1) * 128, :])
                wqk_sb.append(t_wqk)
                t_wv = cp.tile([128, VW], F32R, name=f"wv{c}", tag=f"wv{c}")
                nc.sync.dma_start(t_wv[:], d_wv[c * 128 : (c + 1) * 128, :])
                wv_sb.append(t_wv)
                t_wp = cp.tile([128, C], F32R, name=f"wp{c}", tag=f"wp{c}")
                nc.sync.dma_start(t_wp[:], d_wp[c * 128 : (c + 1) * 128, :])
                wp_sb.append(t_wp)
            bqk_sb = cp.tile([128, QK_TILES], F32, name="bqk", tag="bqk")
            nc.sync.dma_start(bqk_sb[:], d_bqk[:])
            bv_sb = cp.tile([1, VW], F32R, name="bv", tag="bv")
            nc.sync.dma_start(bv_sb[:], d_bv[:])
            bp_sb = cp.tile([1, C], F32R, name="bp", tag="bp")
            nc.sync.dma_start(bp_sb[:], d_bp[:])
            ones_sb = cp.tile([1, 128], F32R, name="ones", tag="ones")
            nc.sync.dma_start(ones_sb[:], d_ones[:])

            def attn_headpair(b, hp, qk_sb, vaug_sb, at_sb, sumsf):
                # head pair (2hp, 2hp+1): even head at partitions 0:64, odd
                # at 64:128 of the same qk tiles. Interleave their score
                # matmuls so consecutive PE ops hit different row groups.
                qt = qk_sb[hp]
                kt = qk_sb[6 + hp]
                pt_sb = {0: [], 1: []}
                for ki, (k0, kl) in enumerate(TCH):
                    q0 = 0 if ki == 0 else 64
                    ps_pair = []
                    for par in range(2):
                        off = par * 64
                        ps = pp.tile(
                            [kl, N - q0],
                            F32,
                            name=f"pst{b}_{hp}_{par}_{ki}",
                            tag="pst",
                            bufs=3,
                        )
                        nc.tensor.matmul(
                            ps[:],
                            kt[off : off + 64, k0 : k0 + kl],
                            qt[off : off + 64, q0:N],
                            start=True,
                            stop=True,
                        )
                        ps_pair.append(ps)
                    for par in range(2):
                        t_pt = wp.tile(
                            [kl, N - q0],
                            F32R,
                            name=f"pt{b}_{hp}_{par}_{ki}",
                            tag="pt",
                            bufs=6,
                        )
                        nc.scalar.activation(t_pt[:], ps_pair[par][:], EXP, scale=0.125)
                        pt_sb[par].append(t_pt)
                for par in range(2):
                    h = 2 * hp + par
                    off = par * 64
                    pts = pt_sb[par]
                    # PV: rows 0:64 = attn outT (unnormalized), row 64 = colsums
                    po = pp.tile([65, N], F32, name=f"po{b}_{h}", tag="po", bufs=3)
                    hs = slice(h * 65, (h + 1) * 65)
                    nc.tensor.matmul(
                        po[:, 0:64],
                        vaug_sb[0][0:64, hs],
                        pts[0][0:64, 0:64],
                        start=True,
                        stop=False,
                    )
                    nc.tensor.matmul(
                        po[:, 64:N],
                        vaug_sb[0][:, hs],
                        pts[0][:, 64:N],
                        start=False,
                        stop=False,
                    )
                    nc.tensor.matmul(
                        po[:, 64:N], vaug_sb[1][:, hs], pts[1][:], start=False, stop=False
                    )
                    nc.tensor.matmul(
                        po[:, 64:N], vaug_sb[2][:, hs], pts[2][:], start=False, stop=True
                    )
                    # evacuate unnormalized rows + colsum row; frees the bank
                    nc.any.tensor_copy(at_sb[h // 2][off : off + 64, :], po[0:64, :])
                    nc.any.tensor_copy(sumsf[0:1, h * N : (h + 1) * N], po[64:65, :])

            def attn_chain(b, sumsf):
                # batched softmax denominators for all 12 heads:
                # scatter the 12 per-head sum rows across partitions (DMA is the
                # only engine free of partition-alignment limits), batch the
                # reciprocal, then flatten back so the broadcast matmuls can
                # slice it at base partition 0
                sums12 = wp.tile([H, N], F32, name=f"sums12{b}", tag="sums12", bufs=1)
                nc.sync.dma_start(sums12[:, :], sumsf[0:1, :])
                rcp = wp.tile([H, N], F32R, name=f"rcp{b}", tag="rcp", bufs=1)
                with nc.allow_low_precision(reason="fp32r reciprocal"):
                    nc.vector.reciprocal(rcp[:], sums12[:])
                rcpf = wp.tile([1, H * N], F32R, name=f"rcpf{b}", tag="rcpf", bufs=1)
                nc.sync.dma_start(rcpf[0:1, :], rcp[:, :])
                return rcpf

            def norm_batch(b, at_sb, rcpf):
                # 12 consecutive rank-1 broadcasts share the ones stationary
                for h in range(H):
                    off = (h % 2) * 64
                    pbc = pp.tile([64, N], F32, name=f"pbc{b}_{h}", tag="pst", bufs=3)
                    nc.tensor.matmul(
                        pbc[:],
                        ones_sb[:, 0:64],
                        rcpf[0:1, h * N : (h + 1) * N],
                        start=True,
                        stop=True,
                    )
                    nc.vector.tensor_mul(
                        at_sb[h // 2][off : off + 64, :],
                        at_sb[h // 2][off : off + 64, :],
                        pbc[:],
                    )

            def proj_unit(b, ti, at_sb):
                t0, tl = TCH[ti]
                t_o = wp.tile([tl, C], F32, name=f"outp{b}_{ti}", tag="outp", bufs=2)
                ps_h = [
                    pp.tile([tl, PH], F32, name=f"psp{b}_{ti}_{nh}", tag="pmm", bufs=2)
                    for nh in range(2)
                ]
                for c in range(CCH):
                    for nh in range(2):
                        nc.tensor.matmul(
                            ps_h[nh][:],
                            at_sb[c][:, t0 : t0 + tl],
                            wp_sb[c][:, nh * PH : (nh + 1) * PH],
                            start=(c == 0),
                            stop=False,
                        )
                for nh in range(2):
                    nc.tensor.matmul(
                        ps_h[nh][:],
                        ones_sb[:, 0:tl],
                        bp_sb[:, nh * PH : (nh + 1) * PH],
                        start=False,
                        stop=True,
                    )
                for nh in range(2):
                    nc.any.tensor_copy(t_o[:, nh * PH : (nh + 1) * PH], ps_h[nh][:])
                nc.sync.dma_start(d_out[b, t0 : t0 + tl, :], t_o[:])

            def vnat_unit(b, ti, xt_sb):
                t0, tl = TCH[ti]
                t_v = wp.tile([tl, VW], F32R, name=f"vaug{b}_{ti}", tag="vaug", bufs=6)
                ps_h = [
                    pp.tile([tl, NPH], F32, name=f"psv{b}_{ti}_{nh}", tag="pmm", bufs=2)
                    for nh in range(2)
                ]
                for c in range(CCH):
                    for nh in range(2):
                        nc.tensor.matmul(
                            ps_h[nh][:],
                            xt_sb[(b, c)][:, t0 : t0 + tl],
                            wv_sb[c][:, nh * NPH : (nh + 1) * NPH],
                            start=(c == 0),
                            stop=False,
                        )
                for nh in range(2):
                    nc.tensor.matmul(
                        ps_h[nh][:],
                        ones_sb[:, 0:tl],
                        bv_sb[:, nh * NPH : (nh + 1) * NPH],
                        start=False,
                        stop=True,
                    )
                for nh in range(2):
                    nc.any.tensor_copy(t_v[:, nh * NPH : (nh + 1) * NPH], ps_h[nh][:])
                return t_v

            def xt_dma(p):
                xt_sb = {}
                for b in (2 * p, 2 * p + 1):
                    for c in range(CCH):
                        t_xt = wp.tile(
                            [128, N], F32R, name=f"xt{b}_{c}", tag="xt", bufs=18
                        )
                        nc.sync.dma_start(t_xt[:], d_xt[b, c * 128 : (c + 1) * 128, :])
                        xt_sb[(b, c)] = t_xt
                return xt_sb

            def qkv_unit(p, j, xt_sb, qk_sb):
                # qkT projection for one co-tile, batch-paired so the weight
                # tile is stationary across two consecutive matmuls
                bpair = (2 * p, 2 * p + 1)
                ps_b = {
                    b: pp.tile([128, N], F32, name=f"psqk{b}_{j}", tag="pmm", bufs=2)
                    for b in bpair
                }
                for c in range(CCH):
                    for b in bpair:
                        nc.tensor.matmul(
                            ps_b[b][:],
                            wqk_sb[c][:, j * 128 : (j + 1) * 128],
                            xt_sb[(b, c)][:],
                            start=(c == 0),
                            stop=(c == CCH - 1),
                        )
                for b in bpair:
                    t_qk = wp.tile([128, N], BF16, name=f"qk{b}_{j}", tag="qkt", bufs=42)
                    with nc.allow_low_precision(reason="bf16 q/k for scores"):
                        nc.vector.tensor_scalar_add(
                            t_qk[:], ps_b[b][:], bqk_sb[:, j : j + 1]
                        )
                    qk_sb[b].append(t_qk)

            def emit_pair(p, qk_sb, filler):
                """vnat + attention + normalize + projection for pair p, with
                filler() hooks where the driver injects the next pair's dense
                qkv matmuls to keep the PE array duty above the HAM clock-gate
                threshold."""
                b0, b1 = 2 * p, 2 * p + 1
                xt_sb = pair_state[p]["xt"]
                vaug0 = [vnat_unit(b0, ti, xt_sb) for ti in range(3)]
                at0 = [
                    wp.tile([128, N], F32R, name=f"at{b0}_{j}", tag="at", bufs=12)
                    for j in range(CCH)
                ]
                sumsf0 = wp.tile([1, H * N], F32, name=f"sumsf{b0}", tag="sumsf", bufs=1)
                vaug1 = []
                for hp in range(H // 2):
                    attn_headpair(b0, hp, qk_sb[b0], vaug0, at0, sumsf0)
                    if hp % 2 == 0:
                        vaug1.append(vnat_unit(b1, hp // 2, xt_sb))
                    else:
                        filler()
                rcpf0 = attn_chain(b0, sumsf0)
                at1 = [
                    wp.tile([128, N], F32R, name=f"at{b1}_{j}", tag="at", bufs=12)
                    for j in range(CCH)
                ]
                sumsf1 = wp.tile([1, H * N], F32, name=f"sumsf{b1}", tag="sumsf", bufs=1)
                last = pair_state[p].get("last", False)
                for hp in range(H // 2):
                    attn_headpair(b1, hp, qk_sb[b1], vaug1, at1, sumsf1)
                    if last:
                        # pipeline drain: no next-pair qkv to inject, so fold
                        # b0's normalize+projection into b1's attention instead
                        if hp == 2:
                            norm_batch(b0, at0, rcpf0)
                        elif hp == 3:
                            proj_unit(b0, 0, at0)
                        elif hp == 4:
                            proj_unit(b0, 1, at0)
                        elif hp == 5:
                            proj_unit(b0, 2, at0)
                    else:
                        filler()
                if not last:
                    norm_batch(b0, at0, rcpf0)
                    filler()
                    proj_unit(b0, 0, at0)
                    filler()
                    proj_unit(b0, 1, at0)
                    filler()
                    proj_unit(b0, 2, at0)
                rcpf1 = attn_chain(b1, sumsf1)
                norm_batch(b1, at1, rcpf1)
                for ti in range(3):
                    proj_unit(b1, ti, at1)

            # ---- software-pipelined driver: pair p's dense qkv matmuls are
            # emitted interleaved into pair p-1's attention phase ----
            NP = BC // 2
            pair_state = {}
            for p in range(NP):
                pair_state[p] = {"qk": {2 * p: [], 2 * p + 1: []}}
            pair_state[0]["xt"] = xt_dma(0)
            for j in range(QK_TILES):
                qkv_unit(0, j, pair_state[0]["xt"], pair_state[0]["qk"])
            for p in range(NP):
                if p + 1 < NP:
                    pair_state[p + 1]["xt"] = xt_dma(p + 1)
                    nxt = pair_state[p + 1]
                    pending = list(range(QK_TILES))

                    def filler(nxt=nxt, pending=pending, pnext=p + 1):
                        if pending:
                            qkv_unit(pnext, pending.pop(0), nxt["xt"], nxt["qk"])
                else:
                    pair_state[p]["last"] = True

                    def filler():
                        pass
                emit_pair(p, pair_state[p]["qk"], filler)

    nc.compile()
    return nc


def _get_nc():
    if "nc" not in _CACHE:
        _CACHE["nc"] = _build()
    return _CACHE["nc"]


def _host_prep(x, w_qkv, b_qkv, w_proj, b_proj):
    x = np.asarray(x, dtype=np.float32)
    w_qkv = np.asarray(w_qkv, dtype=np.float32)
    b_qkv = np.asarray(b_qkv, dtype=np.float32)
    w_proj = np.asarray(w_proj, dtype=np.float32)
    b_proj = np.asarray(b_proj, dtype=np.float32)

    xt = np.ascontiguousarray(x.transpose(0, 2, 1))  # [B, C, N]
    wqk = np.ascontiguousarray(w_qkv[: 2 * C].T)  # [C, 2C]
    wv_nat = w_qkv[2 * C :]  # [C(hd), C(c)]
    wv = np.zeros((C, VW), dtype=np.float32)
    bv = np.zeros((1, VW), dtype=np.float32)
    for h in range(H):
        wv[:, h * 65 : h * 65 + 64] = wv_nat[h * 64 : (h + 1) * 64].T
        bv[0, h * 65 : h * 65 + 64] = b_qkv[2 * C + h * 64 : 2 * C + (h + 1) * 64]
        bv[0, h * 65 + 64] = 1.0
    bqk = np.ascontiguousarray(b_qkv[: 2 * C].reshape(QK_TILES, 128).T)  # [128, 12]
    wpr = np.ascontiguousarray(w_proj.T)  # [C, C]
    bpr = np.ascontiguousarray(b_proj.reshape(1, C))
    ones = np.ones((1, 128), dtype=np.float32)
    return xt, wqk, wv, wpr, bqk, bv, bpr, ones


def _run(x, w_qkv, b_qkv, w_proj, b_proj, trace=False, trace_cores=None):
    from concourse.bass_utils import run_bass_kernel_spmd

    xt, wqk, wv, wpr, bqk, bv, bpr, ones = _host_prep(x, w_qkv, b_qkv, w_proj, b_proj)
    nc = _get_nc()
    in_maps = []
    for i in range(NCORES):
        in_maps.append(
            {
                "xt": xt[i * BC : (i + 1) * BC],
                "wqk": wqk,
                "wv": wv,
                "wp": wpr,
                "bqk": bqk,
                "bv": bv,
                "bp": bpr,
                "ones": ones,
            }
        )
    kwargs = {}
    if trace:
        kwargs = {"trace": True, "trace_cores": trace_cores or [0]}
    res = run_bass_kernel_spmd(nc, in_maps, core_ids=list(range(NCORES)), **kwargs)
    out = np.concatenate([res.results[i]["out"] for i in range(NCORES)], axis=0)
    return out.astype(np.float32), res


def kernel(x, w_qkv, b_qkv, w_proj, b_proj, num_t, num_s):
    assert int(num_t) == NT and int(num_s) == NS
    out, _ = _run(x, w_qkv, b_qkv, w_proj, b_proj)
    return out
